# revision 20
# baseline (speedup 1.0000x reference)
"""Trainium2 Bass kernel for nn_AlignmentMatrix.

Math (per batch b):
    out[b,i,j] = ctx[b]@w1 [i] + asp[b]@w2 [j] + (ctx[b]*w3) @ asp[b].T [i,j]
with ctx [B,L1,H]=[128,1024,600], asp [B,L2,H]=[128,128,600],
w_u=[w1;w2;w3] each [600].

Device-side formulation (all FLOPs on device):
    rhs'[d,j] = w3[d]*asp[b,j,d] + w1[d]        (DVE scale+bias, folds s_ctx)
    s_asp[j]  = sum_d w2[d]*asp[b,j,d]          (thin PE matmuls)
    outT[b,j,i] = sum_d rhs'[d,j]*ctxT[d,i]     (PE, K-chunked)

The contraction is chunked [96,126,126,126,126] so that chunk 0 has a spare
partition row at index 96 (32-aligned, as engine writes require): row 96 of
the chunk-0 stationary holds s_asp (matmul'd straight into PSUM partition 96
via tile_position=(0,96)) and row 96 of the chunk-0 moving ctx is all-ones,
folding the "+ s_asp[j]" term into the main contraction for free -- no
rank-1 matmul per output tile.

Perf design vs the 113us baseline (which was read-DMA + PE co-limited):
  * ctx streams as fp8 E3M4 (1B/elem) -> read bytes drop 22.1->12.3 MB/core.
    The PE upconverts operands to FP22 in normal (non-DoubleRow) matmul
    mode, so e3m4's 4 mantissa bits survive; the stationary rhs' stays fp16
    (mixed-dtype matmul is legal - only fp32 must pair with fp32).  Measured
    on HW: rel err 1.17e-2 vs the 2e-2 gate (e4m3 fails at 2.2e-2).
  * rhs' is built on the DVE (tensor_scalar mult+add with per-partition
    scalar columns), not the ACT engine: the ACT sequencer issues one of
    the two HWDGE read rings and HWDGE DMAs execute FIFO per issuing
    engine, so keeping ACT nearly idle keeps the read stream flowing.  ACT
    only does the PSUM->SBUF output casts.
  * s_asp matmuls are batched over the asp-load slices (N up to 512) and
    write PSUM partition 96 directly, where a lane-aligned DVE copy drops
    them into the stationary fold row.
  * whole-batch ctx DMAs (~5KB contiguous per partition) split across both
    HWDGE rings; asp loads once up front in slices; output writes (fp16)
    via SWDGE so the HWDGE rings carry only reads; the last writes go HWDGE
    to shorten the drain.  Accumulation is fp32 in PSUM.

Sharding: data-parallel over batch, 16 batches per core across 8 cores.
"""

import numpy as np
import ml_dtypes

import concourse.bass as bass
import concourse.bacc as bacc
import concourse.mybir as mybir
import concourse.tile as tile
from concourse.bass_utils import run_bass_kernel_spmd

N_CORES = 8
B = 128
L1 = 1024  # ctx rows (i)
L2 = 128  # asp rows (j)
H = 600  # contraction dim (d)
BPC = B // N_CORES  # batches per core
CS = [96, 126, 126, 126, 126]  # contraction chunk sizes (sum = H)
OFFS = [0, 96, 222, 348, 474]  # chunk offsets into d
KC = len(CS)
KP = 126  # partition rows of the packed chunk layout (max chunk size)
FOLD = 96  # partition row holding the s_asp fold (chunk 0 only)
NI = 512  # moving free-dim per matmul (one fp32 PSUM bank)
NIC = L1 // NI  # i-chunks per batch
OPACK = 1  # batches packed per output DMA
# s_asp matmul groups == asp DMA slices (so no read spans two DMA writes)
ASP_PARTS = [(0, 1), (1, 4), (4, 8), (8, 12), (12, BPC)]

F32 = mybir.dt.float32
F16 = mybir.dt.float16
E3M4 = mybir.dt.float8e3
NP_E3M4 = ml_dtypes.float8_e3m4


def build_kernel():
    nc = bacc.Bacc(
        "TRN2", target_bir_lowering=False, debug=False, enable_asserts=False
    )
    ctx8 = nc.dram_tensor(
        "ctx8", [BPC, KP, KC, L1], E3M4, kind="ExternalInput"
    ).ap()
    aspT = nc.dram_tensor(
        "aspT", [KP, BPC, KC, L2], F16, kind="ExternalInput"
    ).ap()
    wc = nc.dram_tensor("wc", [KP, 2 * KC], F32, kind="ExternalInput").ap()
    w2c = nc.dram_tensor("w2c", [KP, KC], F16, kind="ExternalInput").ap()
    outT = nc.dram_tensor(
        "outT", [BPC // OPACK, L2, OPACK, L1], F16, kind="ExternalOutput"
    ).ap()

    dmae = [nc.sync, nc.scalar]  # the two HWDGE rings

    with tile.TileContext(nc) as tc:
        with (
            tc.tile_pool(name="consts", bufs=1) as consts,
            tc.tile_pool(name="ctx8_pool", bufs=10) as ctx8_pool,
            tc.tile_pool(name="asp_pool", bufs=1) as asp_pool,
            tc.tile_pool(name="rhsp_pool", bufs=3) as rhsp_pool,
            tc.tile_pool(name="out_pool", bufs=2) as out_pool,
            tc.tile_pool(name="ps_out", bufs=4, space="PSUM") as ps_out,
            tc.tile_pool(name="ps_sasp", bufs=2, space="PSUM") as ps_sasp,
        ):
            # First on each ring is a piece of ctx/asp batch 0 so the first
            # real compute is unblocked as early as possible; the tiny
            # wc/w2c loads queue behind it.
            asp_t = asp_pool.tile([KP, BPC, KC, L2], F16)
            nc.sync.dma_start(asp_t[:, 0:1, :, :], aspT[:, 0:1, :, :])
            wc_t = consts.tile([KP, 2 * KC], F32)
            nc.sync.dma_start(wc_t[:], wc[:])
            w2c_t = consts.tile([KP, KC], F16)
            nc.sync.dma_start(w2c_t[:], w2c[:])

            out_sb = None
            sasp_ps = None
            glo = 0
            for b in range(BPC):
                # whole-batch ctx loads, split across the two HWDGE rings
                ctx8_t = ctx8_pool.tile([KP, KC, L1], E3M4, tag="ctx8")
                h = 2
                dmae[(b + 1) % 2].dma_start(
                    ctx8_t[:, 0:h, :], ctx8[b, :, 0:h, :]
                )
                dmae[b % 2].dma_start(
                    ctx8_t[:, h:KC, :], ctx8[b, :, h:KC, :]
                )
                if 1 <= b <= 4:
                    # asp slices ride the SWDGE ring so ctx owns both HWDGE
                    # rings (PE was starving on ctx arrival mid-kernel)
                    lo, hi = ASP_PARTS[b]
                    nc.gpsimd.dma_start(
                        asp_t[:, lo:hi, :, :], aspT[:, lo:hi, :, :]
                    )

                # s_asp[j] = sum_d w2[d]*aspT[d, j] for one asp slice at a
                # time; lands in PSUM partition 96 (tile_position col 96)
                # so the fold-row copy below stays in its lane.
                if b in (lo for lo, _ in ASP_PARTS):
                    glo, ghi = next(
                        p for p in ASP_PARTS if p[0] == b
                    )
                    sasp_ps = ps_sasp.tile(
                        [FOLD + 1, ghi - glo, L2], F32, tag="sasp"
                    )
                    for k in range(KC):
                        nc.tensor.matmul(
                            sasp_ps[FOLD : FOLD + 1, :, :],
                            w2c_t[0 : CS[k], k : k + 1],
                            asp_t[0 : CS[k], glo:ghi, k, :],
                            start=(k == 0),
                            stop=(k == KC - 1),
                            tile_position=(0, FOLD),
                        )

                # rhs'[d, j] = w3[d]*aspT[d, j] + w1[d] on the DVE;
                # row 96 of chunk 0 = s_asp (K-fold of the rank-1 term)
                rhsp = rhsp_pool.tile([KP, KC, L2], F16, tag="rhsp")
                for k in range(KC):
                    nc.vector.tensor_scalar(
                        rhsp[0 : CS[k], k, :],
                        asp_t[0 : CS[k], b, k, :],
                        wc_t[0 : CS[k], KC + k : KC + k + 1],
                        wc_t[0 : CS[k], k : k + 1],
                        mybir.AluOpType.mult,
                        mybir.AluOpType.add,
                    )
                nc.vector.tensor_copy(
                    rhsp[FOLD : FOLD + 1, 0, :],
                    sasp_ps[FOLD : FOLD + 1, b - glo, :],
                )

                if b % OPACK == 0:
                    out_sb = out_pool.tile([L2, OPACK, L1], F16, tag="out")
                for c in range(NIC):
                    out_ps = ps_out.tile([L2, NI], F32, tag="out_ps")
                    for k in range(KC):
                        kp = CS[k] + 1 if k == 0 else CS[k]
                        nc.tensor.matmul(
                            out_ps[:],
                            rhsp[0:kp, k, :],
                            ctx8_t[0:kp, k, c * NI : (c + 1) * NI],
                            start=(k == 0),
                            stop=(k == KC - 1),
                        )
                    nc.scalar.copy(
                        out_sb[:, b % OPACK, c * NI : (c + 1) * NI], out_ps[:]
                    )

                if b % OPACK == OPACK - 1:
                    if b >= BPC - 2:
                        dmae[b % 2].dma_start(outT[b // OPACK], out_sb[:])
                    else:
                        nc.gpsimd.dma_start(outT[b // OPACK], out_sb[:])

    nc.compile()
    return nc


_NC_CACHE = None


def _get_nc():
    global _NC_CACHE
    if _NC_CACHE is None:
        _NC_CACHE = build_kernel()
    return _NC_CACHE


def kernel(batch_size=None, ctx=None, asp=None, w_u=None, **run_kwargs):
    ctx = np.asarray(ctx, dtype=np.float32)
    asp = np.asarray(asp, dtype=np.float32)
    w = np.asarray(w_u, dtype=np.float32).reshape(3, H)

    # pack the contraction dim into [KP=126, KC] chunk columns (chunk 0 has
    # 96 data rows + the all-ones fold row at partition 96)
    wc = np.zeros([KP, 2 * KC], np.float32)
    w2c = np.zeros([KP, KC], np.float16)
    ctx8 = np.zeros([B, KP, KC, L1], NP_E3M4)
    aspT = np.zeros([KP, B, KC, L2], np.float16)
    for k in range(KC):
        lo, cs = OFFS[k], CS[k]
        wc[:cs, k] = w[0, lo : lo + cs]
        wc[:cs, KC + k] = w[2, lo : lo + cs]
        w2c[:cs, k] = w[1, lo : lo + cs]
        ctx8[:, :cs, k, :] = ctx[:, :, lo : lo + cs].transpose(0, 2, 1)
        aspT[:cs, :, k, :] = asp[:, :, lo : lo + cs].transpose(2, 0, 1)
    ctx8[:, FOLD, 0, :] = 1.0  # s_asp fold row

    nc = _get_nc()
    in_maps = [
        {
            "ctx8": ctx8[c * BPC : (c + 1) * BPC],
            "aspT": aspT[:, c * BPC : (c + 1) * BPC],
            "wc": wc,
            "w2c": w2c,
        }
        for c in range(N_CORES)
    ]
    res = run_bass_kernel_spmd(
        nc, in_maps, core_ids=list(range(N_CORES)), **run_kwargs
    )
    outT = np.concatenate(
        [res.results[c]["outT"] for c in range(N_CORES)], axis=0
    ).astype(np.float32)  # [B//OPACK, L2, OPACK, L1]
    out = np.ascontiguousarray(
        outT.transpose(0, 2, 3, 1).reshape(B, L1, L2)
    )  # [B, L1, L2]
    if run_kwargs:
        return out, res
    return out


# revision 21
# speedup vs baseline: 1.0171x; 1.0171x over previous
"""Trainium2 Bass kernel for nn_AlignmentMatrix.

Math (per batch b):
    out[b,i,j] = ctx[b]@w1 [i] + asp[b]@w2 [j] + (ctx[b]*w3) @ asp[b].T [i,j]
with ctx [B,L1,H]=[128,1024,600], asp [B,L2,H]=[128,128,600],
w_u=[w1;w2;w3] each [600].

Device-side formulation (all FLOPs on device):
    rhs'[d,j] = w3[d]*asp[b,j,d] + w1[d]        (DVE scale+bias, folds s_ctx)
    s_asp[j]  = sum_d w2[d]*asp[b,j,d]          (thin PE matmuls)
    outT[b,j,i] = sum_d rhs'[d,j]*ctxT[d,i]     (PE, K-chunked)

The contraction is chunked [96,126,126,126,126] so that chunk 0 has a spare
partition row at index 96 (32-aligned, as engine writes require): row 96 of
the chunk-0 stationary holds s_asp (matmul'd straight into PSUM partition 96
via tile_position=(0,96)) and row 96 of the chunk-0 moving ctx is all-ones,
folding the "+ s_asp[j]" term into the main contraction for free -- no
rank-1 matmul per output tile.

Perf design vs the 113us baseline (which was read-DMA + PE co-limited):
  * ctx streams as fp8 E3M4 (1B/elem) -> read bytes drop 22.1->12.3 MB/core.
    The PE upconverts operands to FP22 in normal (non-DoubleRow) matmul
    mode, so e3m4's 4 mantissa bits survive; the stationary rhs' stays fp16
    (mixed-dtype matmul is legal - only fp32 must pair with fp32).  Measured
    on HW: rel err 1.17e-2 vs the 2e-2 gate (e4m3 fails at 2.2e-2).
  * rhs' is built on the DVE (tensor_scalar mult+add with per-partition
    scalar columns), not the ACT engine: the ACT sequencer issues one of
    the two HWDGE read rings and HWDGE DMAs execute FIFO per issuing
    engine, so keeping ACT nearly idle keeps the read stream flowing.  ACT
    only does the PSUM->SBUF output casts.
  * s_asp matmuls are batched over the asp-load slices (N up to 512) and
    write PSUM partition 96 directly, where a lane-aligned DVE copy drops
    them into the stationary fold row.
  * whole-batch ctx DMAs (~5KB contiguous per partition) split across both
    HWDGE rings; asp loads once up front in slices; output writes (fp16)
    via SWDGE so the HWDGE rings carry only reads; the last writes go HWDGE
    to shorten the drain.  Accumulation is fp32 in PSUM.

Sharding: data-parallel over batch, 16 batches per core across 8 cores.
"""

import numpy as np
import ml_dtypes

import concourse.bass as bass
import concourse.bacc as bacc
import concourse.mybir as mybir
import concourse.tile as tile
from concourse.bass_utils import run_bass_kernel_spmd

N_CORES = 8
B = 128
L1 = 1024  # ctx rows (i)
L2 = 128  # asp rows (j)
H = 600  # contraction dim (d)
BPC = B // N_CORES  # batches per core
CS = [96, 126, 126, 126, 126]  # contraction chunk sizes (sum = H)
OFFS = [0, 96, 222, 348, 474]  # chunk offsets into d
KC = len(CS)
KP = 126  # partition rows of the packed chunk layout (max chunk size)
FOLD = 96  # partition row holding the s_asp fold (chunk 0 only)
NI = 512  # moving free-dim per matmul (one fp32 PSUM bank)
NIC = L1 // NI  # i-chunks per batch
OPACK = 1  # batches packed per output DMA
# s_asp matmul groups == asp DMA slices (so no read spans two DMA writes)
ASP_PARTS = [(0, 1), (1, 4), (4, 8), (8, 12), (12, BPC)]

F32 = mybir.dt.float32
F16 = mybir.dt.float16
E3M4 = mybir.dt.float8e3
NP_E3M4 = ml_dtypes.float8_e3m4


def build_kernel():
    nc = bacc.Bacc(
        "TRN2", target_bir_lowering=False, debug=False, enable_asserts=False
    )
    ctx8 = nc.dram_tensor(
        "ctx8", [BPC, KP, KC, L1], E3M4, kind="ExternalInput"
    ).ap()
    aspT = nc.dram_tensor(
        "aspT", [KP, BPC, KC, L2], F16, kind="ExternalInput"
    ).ap()
    wc = nc.dram_tensor("wc", [KP, 2 * KC], F32, kind="ExternalInput").ap()
    w2c = nc.dram_tensor("w2c", [KP, KC], F16, kind="ExternalInput").ap()
    outT = nc.dram_tensor(
        "outT", [BPC // OPACK, L2, OPACK, L1], F16, kind="ExternalOutput"
    ).ap()

    dmae = [nc.sync, nc.scalar]  # the two HWDGE rings

    with tile.TileContext(nc) as tc:
        with (
            tc.tile_pool(name="consts", bufs=1) as consts,
            tc.tile_pool(name="ctx8_pool", bufs=10) as ctx8_pool,
            tc.tile_pool(name="asp_pool", bufs=1) as asp_pool,
            tc.tile_pool(name="rhsp_pool", bufs=3) as rhsp_pool,
            tc.tile_pool(name="out_pool", bufs=2) as out_pool,
            tc.tile_pool(name="ps_out", bufs=4, space="PSUM") as ps_out,
            tc.tile_pool(name="ps_sasp", bufs=2, space="PSUM") as ps_sasp,
        ):
            # First on each ring is a piece of ctx/asp batch 0 so the first
            # real compute is unblocked as early as possible; the tiny
            # wc/w2c loads queue behind it.
            asp_t = asp_pool.tile([KP, BPC, KC, L2], F16)
            nc.sync.dma_start(asp_t[:, 0:1, :, :], aspT[:, 0:1, :, :])
            wc_t = consts.tile([KP, 2 * KC], F32)
            nc.sync.dma_start(wc_t[:], wc[:])
            w2c_t = consts.tile([KP, KC], F16)
            nc.sync.dma_start(w2c_t[:], w2c[:])

            out_sb = None
            sasp_ps = None
            glo = 0
            for b in range(BPC):
                # whole-batch ctx loads, split across the two HWDGE rings
                ctx8_t = ctx8_pool.tile([KP, KC, L1], E3M4, tag="ctx8")
                h = 2
                dmae[(b + 1) % 2].dma_start(
                    ctx8_t[:, 0:h, :], ctx8[b, :, 0:h, :]
                )
                dmae[b % 2].dma_start(
                    ctx8_t[:, h:KC, :], ctx8[b, :, h:KC, :]
                )
                if 1 <= b <= 4:
                    lo, hi = ASP_PARTS[b]
                    dmae[b % 2].dma_start(
                        asp_t[:, lo:hi, :, :], aspT[:, lo:hi, :, :]
                    )

                # s_asp[j] = sum_d w2[d]*aspT[d, j] for one asp slice at a
                # time; lands in PSUM partition 96 (tile_position col 96)
                # so the fold-row copy below stays in its lane.
                if b in (lo for lo, _ in ASP_PARTS):
                    glo, ghi = next(
                        p for p in ASP_PARTS if p[0] == b
                    )
                    sasp_ps = ps_sasp.tile(
                        [FOLD + 1, ghi - glo, L2], F32, tag="sasp"
                    )
                    for k in range(KC):
                        nc.tensor.matmul(
                            sasp_ps[FOLD : FOLD + 1, :, :],
                            w2c_t[0 : CS[k], k : k + 1],
                            asp_t[0 : CS[k], glo:ghi, k, :],
                            start=(k == 0),
                            stop=(k == KC - 1),
                            tile_position=(0, FOLD),
                        )

                # rhs'[d, j] = w3[d]*aspT[d, j] + w1[d] on the DVE;
                # row 96 of chunk 0 = s_asp (K-fold of the rank-1 term)
                rhsp = rhsp_pool.tile([KP, KC, L2], F16, tag="rhsp")
                for k in range(KC):
                    nc.vector.tensor_scalar(
                        rhsp[0 : CS[k], k, :],
                        asp_t[0 : CS[k], b, k, :],
                        wc_t[0 : CS[k], KC + k : KC + k + 1],
                        wc_t[0 : CS[k], k : k + 1],
                        mybir.AluOpType.mult,
                        mybir.AluOpType.add,
                    )
                nc.vector.tensor_copy(
                    rhsp[FOLD : FOLD + 1, 0, :],
                    sasp_ps[FOLD : FOLD + 1, b - glo, :],
                )

                if b % OPACK == 0:
                    out_sb = out_pool.tile([L2, OPACK, L1], F16, tag="out")
                for c in range(NIC):
                    out_ps = ps_out.tile([L2, NI], F32, tag="out_ps")
                    for k in range(KC):
                        kp = CS[k] + 1 if k == 0 else CS[k]
                        nc.tensor.matmul(
                            out_ps[:],
                            rhsp[0:kp, k, :],
                            ctx8_t[0:kp, k, c * NI : (c + 1) * NI],
                            start=(k == 0),
                            stop=(k == KC - 1),
                        )
                    nc.scalar.copy(
                        out_sb[:, b % OPACK, c * NI : (c + 1) * NI], out_ps[:]
                    )

                if b % OPACK == OPACK - 1:
                    if b >= BPC - 2:
                        dmae[b % 2].dma_start(outT[b // OPACK], out_sb[:])
                    else:
                        nc.gpsimd.dma_start(outT[b // OPACK], out_sb[:])

    nc.compile()
    return nc


_NC_CACHE = None


def _get_nc():
    global _NC_CACHE
    if _NC_CACHE is None:
        _NC_CACHE = build_kernel()
    return _NC_CACHE


def kernel(batch_size=None, ctx=None, asp=None, w_u=None, **run_kwargs):
    ctx = np.asarray(ctx, dtype=np.float32)
    asp = np.asarray(asp, dtype=np.float32)
    w = np.asarray(w_u, dtype=np.float32).reshape(3, H)

    # pack the contraction dim into [KP=126, KC] chunk columns (chunk 0 has
    # 96 data rows + the all-ones fold row at partition 96)
    wc = np.zeros([KP, 2 * KC], np.float32)
    w2c = np.zeros([KP, KC], np.float16)
    ctx8 = np.zeros([B, KP, KC, L1], NP_E3M4)
    aspT = np.zeros([KP, B, KC, L2], np.float16)
    for k in range(KC):
        lo, cs = OFFS[k], CS[k]
        wc[:cs, k] = w[0, lo : lo + cs]
        wc[:cs, KC + k] = w[2, lo : lo + cs]
        w2c[:cs, k] = w[1, lo : lo + cs]
        ctx8[:, :cs, k, :] = ctx[:, :, lo : lo + cs].transpose(0, 2, 1)
        aspT[:cs, :, k, :] = asp[:, :, lo : lo + cs].transpose(2, 0, 1)
    ctx8[:, FOLD, 0, :] = 1.0  # s_asp fold row

    nc = _get_nc()
    in_maps = [
        {
            "ctx8": ctx8[c * BPC : (c + 1) * BPC],
            "aspT": aspT[:, c * BPC : (c + 1) * BPC],
            "wc": wc,
            "w2c": w2c,
        }
        for c in range(N_CORES)
    ]
    res = run_bass_kernel_spmd(
        nc, in_maps, core_ids=list(range(N_CORES)), **run_kwargs
    )
    outT = np.concatenate(
        [res.results[c]["outT"] for c in range(N_CORES)], axis=0
    ).astype(np.float32)  # [B//OPACK, L2, OPACK, L1]
    out = np.ascontiguousarray(
        outT.transpose(0, 2, 3, 1).reshape(B, L1, L2)
    )  # [B, L1, L2]
    if run_kwargs:
        return out, res
    return out


# revision 22
# speedup vs baseline: 1.0847x; 1.0665x over previous
"""Trainium2 Bass kernel for nn_AlignmentMatrix.

Math (per batch b):
    out[b,i,j] = ctx[b]@w1 [i] + asp[b]@w2 [j] + (ctx[b]*w3) @ asp[b].T [i,j]
with ctx [B,L1,H]=[128,1024,600], asp [B,L2,H]=[128,128,600],
w_u=[w1;w2;w3] each [600].

Device-side formulation (all FLOPs on device):
    rhs'[d,j] = w3[d]*asp[b,j,d] + w1[d]        (DVE scale+bias, folds s_ctx)
    s_asp[j]  = sum_d w2[d]*asp[b,j,d]          (thin PE matmuls)
    outT[b,j,i] = sum_d rhs'[d,j]*ctxT[d,i]     (PE, K-chunked)

The contraction is chunked [96,126,126,126,126] so that chunk 0 has a spare
partition row at index 96 (32-aligned, as engine writes require): row 96 of
the chunk-0 stationary holds s_asp (matmul'd straight into PSUM partition 96
via tile_position=(0,96)) and row 96 of the chunk-0 moving ctx is all-ones,
folding the "+ s_asp[j]" term into the main contraction for free -- no
rank-1 matmul per output tile.

Perf design vs the 113us baseline (which was read-DMA + PE co-limited):
  * ctx streams as fp8 E3M4 (1B/elem) -> read bytes drop 22.1->12.3 MB/core.
    The PE upconverts operands to FP22 in normal (non-DoubleRow) matmul
    mode, so e3m4's 4 mantissa bits survive; the stationary rhs' stays fp16
    (mixed-dtype matmul is legal - only fp32 must pair with fp32).  Measured
    on HW: rel err 1.17e-2 vs the 2e-2 gate (e4m3 fails at 2.2e-2).
  * rhs' is built on the DVE (tensor_scalar mult+add with per-partition
    scalar columns), not the ACT engine: the ACT sequencer issues one of
    the two HWDGE read rings and HWDGE DMAs execute FIFO per issuing
    engine, so keeping ACT nearly idle keeps the read stream flowing.  ACT
    only does the PSUM->SBUF output casts.
  * s_asp matmuls are batched over the asp-load slices (N up to 512) and
    write PSUM partition 96 directly, where a lane-aligned DVE copy drops
    them into the stationary fold row.
  * whole-batch ctx DMAs (~5KB contiguous per partition) split across both
    HWDGE rings; asp loads once up front in slices; output writes (fp16)
    via SWDGE so the HWDGE rings carry only reads; the last writes go HWDGE
    to shorten the drain.  Accumulation is fp32 in PSUM.

Sharding: data-parallel over batch, 16 batches per core across 8 cores.
"""

import numpy as np
import ml_dtypes

import concourse.bass as bass
import concourse.bacc as bacc
import concourse.mybir as mybir
import concourse.tile as tile
from concourse.bass_utils import run_bass_kernel_spmd

N_CORES = 8
B = 128
L1 = 1024  # ctx rows (i)
L2 = 128  # asp rows (j)
H = 600  # contraction dim (d)
BPC = B // N_CORES  # batches per core
CS = [96, 126, 126, 126, 126]  # contraction chunk sizes (sum = H)
OFFS = [0, 96, 222, 348, 474]  # chunk offsets into d
KC = len(CS)
KP = 126  # partition rows of the packed chunk layout (max chunk size)
FOLD = 96  # partition row holding the s_asp fold (chunk 0 only)
NI = 512  # moving free-dim per matmul (one fp32 PSUM bank)
NIC = L1 // NI  # i-chunks per batch
OPACK = 2  # batches packed per output DMA
# s_asp matmul groups == asp DMA slices (so no read spans two DMA writes)
ASP_PARTS = [(0, 1), (1, 4), (4, 8), (8, 12), (12, BPC)]

F32 = mybir.dt.float32
F16 = mybir.dt.float16
E3M4 = mybir.dt.float8e3
NP_E3M4 = ml_dtypes.float8_e3m4


def build_kernel():
    nc = bacc.Bacc(
        "TRN2", target_bir_lowering=False, debug=False, enable_asserts=False
    )
    ctx8 = nc.dram_tensor(
        "ctx8", [BPC, KP, KC, L1], E3M4, kind="ExternalInput"
    ).ap()
    aspT = nc.dram_tensor(
        "aspT", [KP, BPC, KC, L2], F16, kind="ExternalInput"
    ).ap()
    wc = nc.dram_tensor("wc", [KP, 2 * KC], F32, kind="ExternalInput").ap()
    w2c = nc.dram_tensor("w2c", [KP, KC], F16, kind="ExternalInput").ap()
    outT = nc.dram_tensor(
        "outT", [BPC // OPACK, L2, OPACK, L1], F16, kind="ExternalOutput"
    ).ap()

    dmae = [nc.sync, nc.scalar]  # the two HWDGE rings

    with tile.TileContext(nc) as tc:
        with (
            tc.tile_pool(name="consts", bufs=1) as consts,
            tc.tile_pool(name="ctx8_pool", bufs=8) as ctx8_pool,
            tc.tile_pool(name="asp_pool", bufs=1) as asp_pool,
            tc.tile_pool(name="rhsp_pool", bufs=3) as rhsp_pool,
            tc.tile_pool(name="out_pool", bufs=2) as out_pool,
            tc.tile_pool(name="ps_out", bufs=4, space="PSUM") as ps_out,
            tc.tile_pool(name="ps_sasp", bufs=2, space="PSUM") as ps_sasp,
        ):
            # First on each ring is a piece of ctx/asp batch 0 so the first
            # real compute is unblocked as early as possible; the tiny
            # wc/w2c loads queue behind it.
            asp_t = asp_pool.tile([KP, BPC, KC, L2], F16)
            nc.sync.dma_start(asp_t[:, 0:1, :, :], aspT[:, 0:1, :, :])
            wc_t = consts.tile([KP, 2 * KC], F32)
            nc.sync.dma_start(wc_t[:], wc[:])
            w2c_t = consts.tile([KP, KC], F16)
            nc.sync.dma_start(w2c_t[:], w2c[:])

            out_sb = None
            sasp_ps = None
            glo = 0
            for b in range(BPC):
                # whole-batch ctx loads, split across the two HWDGE rings
                ctx8_t = ctx8_pool.tile([KP, KC, L1], E3M4, tag="ctx8")
                h = 2
                dmae[(b + 1) % 2].dma_start(
                    ctx8_t[:, 0:h, :], ctx8[b, :, 0:h, :]
                )
                dmae[b % 2].dma_start(
                    ctx8_t[:, h:KC, :], ctx8[b, :, h:KC, :]
                )
                if 1 <= b <= 4:
                    lo, hi = ASP_PARTS[b]
                    dmae[b % 2].dma_start(
                        asp_t[:, lo:hi, :, :], aspT[:, lo:hi, :, :]
                    )

                # s_asp[j] = sum_d w2[d]*aspT[d, j] for one asp slice at a
                # time; lands in PSUM partition 96 (tile_position col 96)
                # so the fold-row copy below stays in its lane.
                if b in (lo for lo, _ in ASP_PARTS):
                    glo, ghi = next(
                        p for p in ASP_PARTS if p[0] == b
                    )
                    sasp_ps = ps_sasp.tile(
                        [FOLD + 1, ghi - glo, L2], F32, tag="sasp"
                    )
                    for k in range(KC):
                        nc.tensor.matmul(
                            sasp_ps[FOLD : FOLD + 1, :, :],
                            w2c_t[0 : CS[k], k : k + 1],
                            asp_t[0 : CS[k], glo:ghi, k, :],
                            start=(k == 0),
                            stop=(k == KC - 1),
                            tile_position=(0, FOLD),
                        )

                # rhs'[d, j] = w3[d]*aspT[d, j] + w1[d] on the DVE;
                # row 96 of chunk 0 = s_asp (K-fold of the rank-1 term)
                rhsp = rhsp_pool.tile([KP, KC, L2], F16, tag="rhsp")
                for k in range(KC):
                    nc.vector.tensor_scalar(
                        rhsp[0 : CS[k], k, :],
                        asp_t[0 : CS[k], b, k, :],
                        wc_t[0 : CS[k], KC + k : KC + k + 1],
                        wc_t[0 : CS[k], k : k + 1],
                        mybir.AluOpType.mult,
                        mybir.AluOpType.add,
                    )
                nc.vector.tensor_copy(
                    rhsp[FOLD : FOLD + 1, 0, :],
                    sasp_ps[FOLD : FOLD + 1, b - glo, :],
                )

                if b % OPACK == 0:
                    out_sb = out_pool.tile([L2, OPACK, L1], F16, tag="out")
                for c in range(NIC):
                    out_ps = ps_out.tile([L2, NI], F32, tag="out_ps")
                    for k in range(KC):
                        kp = CS[k] + 1 if k == 0 else CS[k]
                        nc.tensor.matmul(
                            out_ps[:],
                            rhsp[0:kp, k, :],
                            ctx8_t[0:kp, k, c * NI : (c + 1) * NI],
                            start=(k == 0),
                            stop=(k == KC - 1),
                        )
                    nc.scalar.copy(
                        out_sb[:, b % OPACK, c * NI : (c + 1) * NI], out_ps[:]
                    )

                if b % OPACK == OPACK - 1:
                    if b >= BPC - 4:
                        dmae[(b // OPACK) % 2].dma_start(
                            outT[b // OPACK], out_sb[:]
                        )
                    else:
                        nc.gpsimd.dma_start(outT[b // OPACK], out_sb[:])

    nc.compile()
    return nc


_NC_CACHE = None


def _get_nc():
    global _NC_CACHE
    if _NC_CACHE is None:
        _NC_CACHE = build_kernel()
    return _NC_CACHE


def kernel(batch_size=None, ctx=None, asp=None, w_u=None, **run_kwargs):
    ctx = np.asarray(ctx, dtype=np.float32)
    asp = np.asarray(asp, dtype=np.float32)
    w = np.asarray(w_u, dtype=np.float32).reshape(3, H)

    # pack the contraction dim into [KP=126, KC] chunk columns (chunk 0 has
    # 96 data rows + the all-ones fold row at partition 96)
    wc = np.zeros([KP, 2 * KC], np.float32)
    w2c = np.zeros([KP, KC], np.float16)
    ctx8 = np.zeros([B, KP, KC, L1], NP_E3M4)
    aspT = np.zeros([KP, B, KC, L2], np.float16)
    for k in range(KC):
        lo, cs = OFFS[k], CS[k]
        wc[:cs, k] = w[0, lo : lo + cs]
        wc[:cs, KC + k] = w[2, lo : lo + cs]
        w2c[:cs, k] = w[1, lo : lo + cs]
        ctx8[:, :cs, k, :] = ctx[:, :, lo : lo + cs].transpose(0, 2, 1)
        aspT[:cs, :, k, :] = asp[:, :, lo : lo + cs].transpose(2, 0, 1)
    ctx8[:, FOLD, 0, :] = 1.0  # s_asp fold row

    nc = _get_nc()
    in_maps = [
        {
            "ctx8": ctx8[c * BPC : (c + 1) * BPC],
            "aspT": aspT[:, c * BPC : (c + 1) * BPC],
            "wc": wc,
            "w2c": w2c,
        }
        for c in range(N_CORES)
    ]
    res = run_bass_kernel_spmd(
        nc, in_maps, core_ids=list(range(N_CORES)), **run_kwargs
    )
    outT = np.concatenate(
        [res.results[c]["outT"] for c in range(N_CORES)], axis=0
    ).astype(np.float32)  # [B//OPACK, L2, OPACK, L1]
    out = np.ascontiguousarray(
        outT.transpose(0, 2, 3, 1).reshape(B, L1, L2)
    )  # [B, L1, L2]
    if run_kwargs:
        return out, res
    return out
